# revision 2
# baseline (speedup 1.0000x reference)
"""Multi-head causal self-attention (B=4, T=2048, C=1024, 16 heads) on 8 TRN2 cores.

Sharding: core i -> batch b = i//2, head-group g = i%2 (8 heads each).
Per core: column-parallel QKV, per-head causal attention, row-parallel proj
producing a partial output; the host sums the two partials per batch + bias.

Kernel math (per core, fp32 throughout):
  A: x [T,C] is PE-transposed to xT [C,T]  (fp32 has no DMA-transpose path)
  B: Q^T,K^T [feat,T] = (w_q|w_k as lhsT) @ xT ; V [T,feat] = xT.T @ w_v.
     V is stored with an extra ones-column per head (65 wide).
  C: per head h, per 512-query chunk c:
       S^T[128k,512q] = K^T_h(j-block) . Q^T_h(chunk)   (K=64 contraction)
       + additive causal mask on the 4 diagonal blocks, exp on ACT
       O^T[65,512] += [V_h|1]^T . P^T   accumulated over j  (row 64 = softmax denom L)
     normalize with reciprocal(L) broadcast over partitions (GPSIMD).
  D: y_partial[T,C] = O^T as lhsT @ w_proj rows (row-parallel) -> DMA out.
"""

import numpy as np

B, T, C = 4, 2048, 1024
H, HD = 16, 64
NCORES = 8
HL = H // 2  # heads per core
DL = HL * HD  # 512 local features
KC = C // 128  # 8 contraction chunks
TB = T // 128  # 16 row blocks
TQ = T // 512  # 4 query chunks
MASK_VAL = -1e30
SCALE = 1.0 / np.sqrt(HD)


def _build_nc(n_iters=1):
    from contextlib import ExitStack, nullcontext

    import concourse.mybir as mybir
    import concourse.tile as tile
    from concourse import bacc
    from concourse.bass import ts
    from concourse.masks import make_identity

    f32 = mybir.dt.float32
    Exp = mybir.ActivationFunctionType.Exp

    nc = bacc.Bacc("TRN2", target_bir_lowering=False, debug=False)
    x_d = nc.dram_tensor("x", [T, C], f32, kind="ExternalInput").ap()
    wqkv_d = nc.dram_tensor("wqkv", [C, 3 * DL], f32, kind="ExternalInput").ap()
    wproj_d = nc.dram_tensor("wproj", [DL, C], f32, kind="ExternalInput").ap()
    y_d = nc.dram_tensor("y", [T, C], f32, kind="ExternalOutput").ap()

    with tile.TileContext(nc) as tc:
        loop_ctx = tc.For_i(0, n_iters, 1) if n_iters > 1 else nullcontext()
        with loop_ctx, ExitStack() as root:
            const = root.enter_context(tc.tile_pool(name="const", bufs=1))
            identity = const.tile([128, 128], f32)
            make_identity(nc, identity)
            # masks[:, m, :]: keep (0) where q_local >= 128m + k_local else -1e30
            masks = const.tile([128, 4, 512], f32)
            for m in range(4):
                nc.gpsimd.memset(masks[:, m, :], 0.0)
                nc.gpsimd.affine_select(
                    out=masks[:, m, :],
                    in_=masks[:, m, :],
                    compare_op=mybir.AluOpType.is_ge,
                    fill=MASK_VAL,
                    base=-128 * m,
                    channel_multiplier=-1,
                    pattern=[[1, 512]],
                )

            persist = root.enter_context(tc.tile_pool(name="persist", bufs=1))
            qt_kt = persist.tile([128, 8, T], f32)  # blocks 0..3 Q^T, 4..7 K^T
            v_sb = persist.tile([128, TB, HL * 65], f32)
            nc.vector.memset(
                v_sb.rearrange("p j (h e) -> p j h e", e=65)[:, :, :, 64:65], 1.0
            )

            # ---- stages A (transpose x) + B (QKV) ----
            with ExitStack() as sAB:
                wq_pool = sAB.enter_context(tc.tile_pool(name="wq", bufs=1))
                wqkv_sb = wq_pool.tile([128, KC, 3 * DL], f32)
                nc.sync.dma_start(
                    wqkv_sb[:], wqkv_d.rearrange("(ko p) n -> p ko n", p=128)
                )
                xload = sAB.enter_context(tc.tile_pool(name="xload", bufs=3))
                xt_pool = sAB.enter_context(tc.tile_pool(name="xt", bufs=1))
                psAB = sAB.enter_context(
                    tc.tile_pool(name="psAB", bufs=2, space="PSUM")
                )
                for c4 in range(TQ):
                    xt = xt_pool.tile([128, KC, 512], f32)
                    for tb in range(4):
                        jb = 4 * c4 + tb
                        xl = xload.tile([128, C], f32)
                        nc.sync.dma_start(xl[:], x_d[ts(jb, 128), :])
                        for k in range(KC):
                            ps = psAB.tile([128, 128], f32, tag="tr")
                            nc.tensor.transpose(ps[:], xl[:, ts(k, 128)], identity[:])
                            nc.scalar.copy(xt[:, k, ts(tb, 128)], ps[:])
                    # Q^T (f 0..3) and K^T (f 4..7) feature blocks for this chunk
                    for f in range(8):
                        ps = psAB.tile([128, 512], f32, tag="mm")
                        for k in range(KC):
                            nc.tensor.matmul(
                                ps[:],
                                wqkv_sb[:, k, ts(f, 128)],
                                xt[:, k, :],
                                start=(k == 0),
                                stop=(k == KC - 1),
                            )
                        nc.vector.tensor_copy(qt_kt[:, f, ts(c4, 512)], ps[:])
                    # V row-blocks
                    for tb in range(4):
                        jb = 4 * c4 + tb
                        ps = psAB.tile([128, 512], f32, tag="mm")
                        for k in range(KC):
                            nc.tensor.matmul(
                                ps[:],
                                xt[:, k, ts(tb, 128)],
                                wqkv_sb[:, k, 2 * DL : 3 * DL],
                                start=(k == 0),
                                stop=(k == KC - 1),
                            )
                        nc.vector.tensor_copy(
                            v_sb[:, jb, :].rearrange("p (h e) -> p h e", e=65)[
                                :, :, 0:64
                            ],
                            ps.rearrange("p (h e) -> p h e", e=64),
                        )

            # ---- stages C (attention) + D (proj) ----
            with ExitStack() as sCD:
                late = sCD.enter_context(tc.tile_pool(name="late", bufs=1))
                wproj_sb = late.tile([128, DL // 128, C], f32)
                nc.sync.dma_start(
                    wproj_sb[:], wproj_d.rearrange("(ko p) n -> p ko n", p=128)
                )
                ot_sb = late.tile([128, DL // 128, T], f32)

                with ExitStack() as sC:
                    pt_pool = sC.enter_context(tc.tile_pool(name="pt", bufs=4))
                    nrm = sC.enter_context(tc.tile_pool(name="nrm", bufs=3))
                    psS = sC.enter_context(
                        tc.tile_pool(name="psS", bufs=3, space="PSUM")
                    )
                    psOT = sC.enter_context(
                        tc.tile_pool(name="psOT", bufs=2, space="PSUM")
                    )
                    for h in range(HL):
                        f, half = divmod(h, 2)
                        po = 64 * half
                        for c in range(TQ):
                            jmax = 4 * c + 3
                            ot_ps = psOT.tile([65, 512], f32)
                            for j in range(jmax + 1):
                                s_ps = psS.tile([128, 512], f32)
                                nc.tensor.matmul(
                                    s_ps[:],
                                    qt_kt[po : po + 64, 4 + f, ts(j, 128)],
                                    qt_kt[po : po + 64, f, ts(c, 512)],
                                    start=True,
                                    stop=True,
                                )
                                m = j - 4 * c
                                if m >= 0:
                                    nc.vector.tensor_add(
                                        s_ps[:], s_ps[:], masks[:, m, :]
                                    )
                                pt = pt_pool.tile([128, 512], f32)
                                nc.scalar.activation(
                                    pt[:], s_ps[:], Exp, scale=float(SCALE)
                                )
                                nc.tensor.matmul(
                                    ot_ps[:],
                                    v_sb[:, j, ts(h, 65)],
                                    pt[:],
                                    start=(j == 0),
                                    stop=(j == jmax),
                                )
                            recip = nrm.tile([1, 512], f32, tag="recip")
                            nc.vector.reciprocal(recip[:], ot_ps[64:65, :])
                            bc = nrm.tile([128, 512], f32, tag="bc")
                            nc.gpsimd.partition_broadcast(bc[:], recip[:])
                            nc.vector.tensor_mul(
                                ot_sb[po : po + 64, f, ts(c, 512)],
                                ot_ps[0:64, :],
                                bc[0:64, :],
                            )

                with ExitStack() as sD:
                    y_pool = sD.enter_context(tc.tile_pool(name="y", bufs=3))
                    psD = sD.enter_context(
                        tc.tile_pool(name="psD", bufs=2, space="PSUM")
                    )
                    for tb in range(TB):
                        for nn in range(2):
                            ps = psD.tile([128, 512], f32)
                            for f in range(DL // 128):
                                nc.tensor.matmul(
                                    ps[:],
                                    ot_sb[:, f, ts(tb, 128)],
                                    wproj_sb[:, f, ts(nn, 512)],
                                    start=(f == 0),
                                    stop=(f == DL // 128 - 1),
                                )
                            yt = y_pool.tile([128, 512], f32)
                            nc.vector.tensor_copy(yt[:], ps[:])
                            nc.sync.dma_start(y_d[ts(tb, 128), ts(nn, 512)], yt[:])

    nc.compile()
    return nc


def _shard_inputs(x, w_qkv, w_proj):
    """Per-core input dicts: core i -> batch i//2, head-group i%2."""
    in_maps = []
    for i in range(NCORES):
        b, g = divmod(i, 2)
        cols = slice(DL * g, DL * (g + 1))
        wqkv_local = np.concatenate(
            [w_qkv[:, cols], w_qkv[:, 1024:2048][:, cols], w_qkv[:, 2048:3072][:, cols]],
            axis=1,
        )
        in_maps.append(
            {
                "x": np.ascontiguousarray(x[b]),
                "wqkv": np.ascontiguousarray(wqkv_local),
                "wproj": np.ascontiguousarray(w_proj[DL * g : DL * (g + 1), :]),
            }
        )
    return in_maps


_cached_nc = None


def kernel(x, w_qkv, w_proj, b_proj):
    global _cached_nc
    from concourse.bass_utils import run_bass_kernel_spmd

    x = np.asarray(x, dtype=np.float32)
    w_qkv = np.asarray(w_qkv, dtype=np.float32)
    w_proj = np.asarray(w_proj, dtype=np.float32)
    b_proj = np.asarray(b_proj, dtype=np.float32)

    if _cached_nc is None:
        _cached_nc = _build_nc()

    in_maps = _shard_inputs(x, w_qkv, w_proj)
    res = run_bass_kernel_spmd(_cached_nc, in_maps, core_ids=list(range(NCORES)))

    out = np.empty((B, T, C), dtype=np.float32)
    for b in range(B):
        out[b] = res.results[2 * b]["y"] + res.results[2 * b + 1]["y"]
    out += b_proj[None, None, :]
    return out
